# revision 2
# baseline (speedup 1.0000x reference)
# Neural CDE kernel for 8 Trainium2 NeuronCores — v4.
# Data-parallel over batch: 4096 -> 512/core; 255-step RK4 scan local per core.
#
# v4: xd (spline derivative at the 3 RK4 eval points per step) is z-independent,
# so it is precomputed on the HOST into a DRAM table of [1, C*BC] bf16 rows and
# DMA-broadcast to [H, C*BC] SBUF tiles, prefetched 2 steps ahead on the SP
# queue. This removes the per-step SBUF->DRAM->broadcast staging chain and the
# sync-queue serialization that dominated v3. coeffs input is replaced by the
# xd table + pre-transposed a0.
#
# Retained from v3: full-width [H, 512] stages; c-einsum via tanh pieces (bf16)
# * xd (DVE) summed on PE via identity-matmul PSUM accumulation; RK4 y-combines
# folded into mm1 PSUM groups with scaled W1 copies; softplus = Abs, Exp (ACT)
# + fused DVE relu(x)+poly3(log1p(e)); layer-1 bias rides in u = W1 z + b1.
import sys
sys.path.insert(0, '/opt/trn_rl_repo')

import numpy as np

B_FULL, N_INT, C, H = 4096, 255, 8, 128
N_CORES = 8
B_CORE = B_FULL // N_CORES          # 512
BC = B_CORE
N_STEPS_DEFAULT = 255

# log1p(t) ~ c1 t + c2 t^2 + c3 t^3 on (0,1], max abs err 5.4e-4
SC1, SC2, SC3 = 0.9874542, -0.40841109, 0.11463897

# where the Abs of softplus runs: 'act' (scalar engine, v3 behavior),
# 'dve' (vector engine stt), 'pool' (gpsimd stt)
ABS_ENGINE = 'act'

_REGISTERED = {}


def _register_ops():
    if _REGISTERED:
        return _REGISTERED
    import concourse.dve_ops as dve_ops
    from concourse.dve_spec import (Spec, Src0, Src1, C0, C1, C2, Zero,
                                    maxx, lower, _has_src1)
    from concourse.dve_uop import DveOpSpec

    def reg(name, spec):
        if name in dve_ops._SUB_OPCODE_FOR_NAME:
            return next(o for o in dve_ops.OPS if o.name == name)
        shas = {}
        for ver in ("v3", "v4"):
            s = DveOpSpec(name=name, opcode=0, uops=lower(spec, ver=ver),
                          rd1_en=_has_src1(spec))
            shas[ver] = s.sha(ver)
        op = dve_ops.DveOp(name, spec, False, uops_sha=shas)
        dve_ops.OPS.append(op)
        dve_ops.CUSTOM_DVE_SPECS[name] = spec
        dve_ops._SUB_OPCODE_FOR_NAME[name] = max(
            dve_ops._SUB_OPCODE_FOR_NAME.values()) + 1
        return op

    def _sptail_ref(in0, in1, s0, s1, imm2):
        x = in0.astype(np.float32)
        t = in1.astype(np.float32)
        return np.maximum(x, 0) + ((imm2 * t + s1) * t + s0) * t

    _REGISTERED['SPTAIL3'] = reg(
        "SPTAIL3_ANT",
        Spec(body=maxx(Src0, Zero) + (((Src1 * C2 + C1) * Src1) + C0) * Src1,
             reference=_sptail_ref))
    return _REGISTERED


_NC_CACHE = {}


def build(n_steps=N_STEPS_DEFAULT):
    key = n_steps
    if key in _NC_CACHE:
        return _NC_CACHE[key]
    import concourse.mybir as mybir
    import concourse.tile as tile
    from concourse import bacc

    ops = _register_ops()
    SPTAIL3 = ops['SPTAIL3']

    f32 = mybir.dt.float32
    f32r = mybir.dt.float32r
    bf16 = mybir.dt.bfloat16
    AF = mybir.ActivationFunctionType
    OP = mybir.AluOpType
    OPm, OPa = OP.mult, OP.add

    nc = bacc.Bacc()
    # xd table: rows 3*s+j for j in {0:frac=1/3, 1:frac=2/3, 2:frac=1},
    # plus final row = xd(step 0, frac 0) for the very first k1.
    xdt_d = nc.dram_tensor("xdt", [3 * n_steps + 1, C * BC], bf16,
                           kind="ExternalInput")
    wd = {}
    for nm in ["W1", "W1_3", "W1_m3", "W1_m", "W1_8", "I128", "W2"]:
        wd[nm] = nc.dram_tensor(nm, [H, H], f32, kind="ExternalInput")
    W3s_d = nc.dram_tensor("W3s", [H, H * C], f32, kind="ExternalInput")
    b3c_d = nc.dram_tensor("b3c", [H, C], f32, kind="ExternalInput")
    b1r_d = nc.dram_tensor("b1r", [1, H], f32, kind="ExternalInput")
    b2r_d = nc.dram_tensor("b2r", [1, H], f32, kind="ExternalInput")
    a0_d = nc.dram_tensor("a0", [C, BC], f32, kind="ExternalInput")
    Wi_d = nc.dram_tensor("Wi", [C, H], f32, kind="ExternalInput")
    bi_d = nc.dram_tensor("bic", [H, 1], f32, kind="ExternalInput")
    Wo_d = nc.dram_tensor("Wo", [H, 1], f32, kind="ExternalInput")
    bo_d = nc.dram_tensor("bo", [1, 1], f32, kind="ExternalInput")
    ones_d = nc.dram_tensor("ones", [1, BC], f32, kind="ExternalInput")
    out_d = nc.dram_tensor("out", [BC, 1], f32, kind="ExternalOutput")

    with tile.TileContext(nc) as tc:
        with tc.tile_pool(name="const", bufs=1) as cp, \
             tc.tile_pool(name="zpool", bufs=2) as zp, \
             tc.tile_pool(name="work", bufs=2) as wp, \
             tc.tile_pool(name="tanh", bufs=6) as thp, \
             tc.tile_pool(name="mul", bufs=6) as mp, \
             tc.tile_pool(name="kpool", bufs=2) as kp, \
             tc.tile_pool(name="xdrep", bufs=4) as xp, \
             tc.tile_pool(name="psA", bufs=2, space="PSUM") as psA, \
             tc.tile_pool(name="psK", bufs=1, space="PSUM") as psK, \
             tc.tile_pool(name="ps3", bufs=5, space="PSUM") as ps3:

            # ---- constants ----
            W1_t = cp.tile([H, H], f32r, tag="w1")
            W1_3_t = cp.tile([H, H], f32r, tag="w13")
            W1_m3_t = cp.tile([H, H], f32r, tag="w1m3")
            W1_m_t = cp.tile([H, H], f32r, tag="w1m")
            W1_8_t = cp.tile([H, H], f32r, tag="w18")
            I_t = cp.tile([H, H], f32r, tag="ieye")
            Ib_t = cp.tile([H, H], bf16, tag="ibf")
            W2_t = cp.tile([H, H], f32r, tag="w2")
            W3_t = cp.tile([H, H * C], f32r, tag="w3")
            b3c_t = cp.tile([H, C], f32, tag="b3c")
            W1bf_t = cp.tile([H, H], bf16, tag="w1bf")
            W13bf_t = cp.tile([H, H], bf16, tag="w13bf")
            W18bf_t = cp.tile([H, H], bf16, tag="w18bf")
            b1r_t = cp.tile([1, H], f32r, tag="b1r")
            b2r_t = cp.tile([1, H], f32r, tag="b2r")
            Wi_t = cp.tile([C, H], f32r, tag="wi")
            Wo_t = cp.tile([H, 1], f32r, tag="wo")
            bi_t = cp.tile([H, 1], f32, tag="bi")
            bo_t = cp.tile([1, 1], f32, tag="bo")
            a0_t = cp.tile([C, BC], f32r, tag="a0")
            ones_t = cp.tile([1, BC], f32r, tag="ones")
            for t_, d_ in [(W1_t, wd["W1"]), (W1_3_t, wd["W1_3"]),
                           (W1_m3_t, wd["W1_m3"]), (W1_m_t, wd["W1_m"]),
                           (W1_8_t, wd["W1_8"]), (I_t, wd["I128"]),
                           (Ib_t, wd["I128"]), (W2_t, wd["W2"]),
                           (W3_t, W3s_d),
                           (W1bf_t, wd["W1"]), (W13bf_t, wd["W1_3"]),
                           (W18bf_t, wd["W1_8"]),
                           (b1r_t, b1r_d), (b2r_t, b2r_d), (Wi_t, Wi_d),
                           (Wo_t, Wo_d), (a0_t, a0_d)]:
                nc.gpsimd.dma_start(t_[:], d_[:])
            for t_, d_ in [(bi_t, bi_d), (bo_t, bo_d), (b3c_t, b3c_d)]:
                nc.sync.dma_start(t_[:], d_[:])
            nc.gpsimd.dma_start(ones_t[:], ones_d[:])

            # ---- z0 = a0 @ Wi + bi  (feature-major [H, B]) ----
            zps = psA.tile([H, BC], f32, tag="pA")
            nc.tensor.matmul(zps[:], Wi_t[:], a0_t[:], start=True, stop=True)
            zT = zp.tile([H, BC], f32r, tag="z")
            nc.scalar.activation(zT[:], zps[:], AF.Identity, bias=bi_t[:])

            # ---- xd loading: broadcast one DRAM row across 128 partitions ----
            def load_xd(row, tag):
                rep = xp.tile([H, C, BC], bf16, tag=tag)
                nc.sync.dma_start(
                    rep[:].rearrange("h s b -> h (s b)"),
                    xdt_d[row:row + 1, :].to_broadcast((H, C * BC)))
                return rep

            def load_step(step):
                return [load_xd(3 * step + j, f"x{j}") for j in range(3)]

            xd_prev = load_xd(3 * n_steps, "x0")
            pend = {0: load_step(0)}
            if n_steps > 1:
                pend[1] = load_step(1)

            # ---- u0 = W1 z0 + b1 ; p1 seeded for step-0 k1 ----
            p1_t = psA.tile([H, BC], f32, tag="pA", name="p1_init")
            nc.tensor.matmul(p1_t[:], b1r_t[:], ones_t[:],
                             start=True, stop=False)
            nc.tensor.matmul(p1_t[:], W1_t[:], zT[:], start=False, stop=True)
            u_t = wp.tile([H, BC], f32r, tag="u", name="u_init")
            nc.scalar.activation(u_t[:], p1_t[:], AF.Copy)

            def softplus_head(p1):
                """e = exp(-|p1|) on the chosen engine split."""
                e1 = wp.tile([H, BC], f32, tag="spe")
                if ABS_ENGINE == 'act':
                    a1 = wp.tile([H, BC], f32, tag="spa")
                    nc.scalar.activation(a1[:], p1[:], AF.Abs)
                    nc.scalar.activation(e1[:], a1[:], AF.Exp, scale=-1.0)
                else:
                    eng = nc.vector if ABS_ENGINE == 'dve' else nc.gpsimd
                    n1 = wp.tile([H, BC], f32, tag="spa")
                    eng.scalar_tensor_tensor(n1[:], p1[:], -1.0, p1[:],
                                             OPm, OP.min)
                    nc.scalar.activation(e1[:], n1[:], AF.Exp)
                return e1

            def tail(p1, xd_rep, j, feed=None):
                """softplus -> mm2 -> softplus -> mm3 (8 single-slot pieces)
                -> tanh -> mult (DVE) -> PE sum -> k."""
                e1 = softplus_head(p1)
                h1 = wp.tile([H, BC], f32r, tag="spha")
                nc.vector._custom_dve(SPTAIL3, out=h1[:], in0=p1[:],
                                      in1=e1[:], s0=SC1, s1=SC2, imm2=SC3)
                p2 = psA.tile([H, BC], f32, tag="pA")
                nc.tensor.matmul(p2[:], b2r_t[:], ones_t[:],
                                 start=True, stop=False)
                nc.tensor.matmul(p2[:], W2_t[:], h1[:], start=False, stop=True)
                e2 = softplus_head(p2)
                h2 = wp.tile([H, BC], f32r, tag="sphb")
                nc.vector._custom_dve(SPTAIL3, out=h2[:], in0=p2[:],
                                      in1=e2[:], s0=SC1, s1=SC2, imm2=SC3)
                # mm3: 8 single-slot pieces; b3 folded into tanh bias
                p3s = []
                for slot in range(C):
                    p3 = ps3.tile([H, BC], f32, tag="p3")
                    nc.tensor.matmul(p3[:],
                                     W3_t[:, slot * H:(slot + 1) * H], h2[:],
                                     start=True, stop=True)
                    p3s.append(p3)
                Ts = []
                for slot in range(C):
                    T_t = thp.tile([H, BC], bf16, tag="T")
                    nc.scalar.activation(T_t[:], p3s[slot][:], AF.Tanh,
                                         bias=b3c_t[:, slot:slot + 1])
                    Ts.append(T_t)
                pK = psK.tile([H, BC], f32, tag="pK")
                for slot in range(C):
                    P_t = mp.tile([H, BC], bf16, tag="P")
                    nc.vector.tensor_tensor(P_t[:], Ts[slot][:],
                                            xd_rep[:, slot, :], OPm)
                    nc.tensor.matmul(pK[:], Ib_t[:], P_t[:],
                                     start=(slot == 0), stop=(slot == C - 1))
                    if feed is not None:
                        Wf, pn = feed
                        nc.tensor.matmul(pn[:], Wf[:], P_t[:],
                                         start=False, stop=(slot == C - 1))
                k_t = kp.tile([H, BC], f32r, tag=f"k{j}")
                nc.vector.tensor_scalar_add(k_t[:], pK[:], 0.0)
                return k_t

            for step in range(n_steps):
                if step + 2 < n_steps:
                    pend[step + 2] = load_step(step + 2)
                xd13, xd23, xd1 = pend.pop(step)

                # k2's p1 group opens first: I u, then dirP W1/3 P(k1)
                pn2 = psA.tile([H, BC], f32, tag="pA", name="pn2")
                nc.tensor.matmul(pn2[:], I_t[:], u_t[:],
                                 start=True, stop=False)
                k1 = tail(p1_t, xd_prev, 1, feed=(W13bf_t, pn2))

                pn3 = psA.tile([H, BC], f32, tag="pA", name="pn3")
                nc.tensor.matmul(pn3[:], I_t[:], u_t[:],
                                 start=True, stop=False)
                nc.tensor.matmul(pn3[:], W1_m3_t[:], k1[:],
                                 start=False, stop=False)
                k2 = tail(pn2, xd13, 2, feed=(W1bf_t, pn3))

                w1 = wp.tile([H, BC], f32, tag="w1t")
                nc.vector.scalar_tensor_tensor(w1[:], k2[:], 3.0, k1[:],
                                               OPm, OPa)

                pn4 = psA.tile([H, BC], f32, tag="pA", name="pn4")
                nc.tensor.matmul(pn4[:], I_t[:], u_t[:],
                                 start=True, stop=False)
                nc.tensor.matmul(pn4[:], W1_t[:], k1[:],
                                 start=False, stop=False)
                nc.tensor.matmul(pn4[:], W1_m_t[:], k2[:],
                                 start=False, stop=False)
                k3 = tail(pn3, xd23, 3, feed=(W1bf_t, pn4))

                w2 = wp.tile([H, BC], f32r, tag="w2t")
                nc.vector.scalar_tensor_tensor(w2[:], k3[:], 3.0, w1[:],
                                               OPm, OPa)

                last = step == n_steps - 1
                if not last:
                    pn1 = psA.tile([H, BC], f32, tag="pA", name="pn1")
                    nc.tensor.matmul(pn1[:], I_t[:], u_t[:],
                                     start=True, stop=False)
                    nc.tensor.matmul(pn1[:], W1_8_t[:], w2[:],
                                     start=False, stop=False)
                    k4 = tail(pn4, xd1, 4, feed=(W18bf_t, pn1))
                else:
                    k4 = tail(pn4, xd1, 4)

                # z' = (z + w2/8) + k4/8
                zn = zp.tile([H, BC], f32r, tag="z")
                w3_ = wp.tile([H, BC], f32r, tag="w3t")
                nc.vector.scalar_tensor_tensor(w3_[:], w2[:], 0.125, zT[:],
                                               OPm, OPa)
                nc.vector.scalar_tensor_tensor(zn[:], k4[:], 0.125, w3_[:],
                                               OPm, OPa)
                if not last:
                    p1_t = pn1
                    u_t = wp.tile([H, BC], f32r, tag="u")
                    nc.scalar.activation(u_t[:], pn1[:], AF.Copy)
                zT = zn
                xd_prev = xd1

            # ---- out = zT @ W_out + b_out ----
            ops_ = psK.tile([H, BC], f32, tag="pK")
            nc.tensor.matmul(ops_[0:1, :], Wo_t[:], zT[:],
                             start=True, stop=True)
            ot = cp.tile([1, BC], f32, tag="outs")
            nc.scalar.activation(ot[:], ops_[0:1, :], AF.Identity,
                                 bias=bo_t[:])
            nc.sync.dma_start(
                out_d[:].rearrange("(p bh) one -> one (p bh)", p=H), ot[:])

    nc.finalize()
    _NC_CACHE[key] = nc
    return nc


def host_inputs(inputs, core, n_steps=N_STEPS_DEFAULT):
    import ml_dtypes
    coeffs = np.asarray(inputs["coeffs"][core * BC:(core + 1) * BC],
                        dtype=np.float32)          # [BC, N_INT, 4C]
    bco = coeffs[:, :, C:2 * C]                     # [BC, n, C]
    two_c = coeffs[:, :, 2 * C:3 * C]
    three_d = coeffs[:, :, 3 * C:4 * C]
    # xd rows: for step s, j in {0,1,2} -> frac (j+1)/3; last row: step0 frac0.
    fr = (np.arange(1, 4, dtype=np.float32) / 3.0)[None, :, None, None]
    xd = bco[:, None] + (two_c[:, None] + three_d[:, None] * fr) * fr
    # [BC, 3, n, C] -> rows [n*3, C*BC] with c-major, batch natural order
    xd = xd[:, :, :n_steps, :].transpose(2, 1, 3, 0)   # [n, 3, C, BC]
    xdt = np.concatenate(
        [xd.reshape(3 * n_steps, C * BC), bco[:, 0, :].T.reshape(1, C * BC)],
        axis=0)
    W1 = inputs["W1"].astype(np.float32)
    W3 = inputs["W3"].astype(np.float32)
    # W3s[h_in, slot*H + h_out] = W3[h_in, h_out*C + slot]
    W3s = np.ascontiguousarray(
        W3.reshape(H, H, C).transpose(0, 2, 1).reshape(H, H * C))
    b3c = np.ascontiguousarray(
        inputs["b3"].astype(np.float32).reshape(H, C))
    return dict(
        xdt=np.ascontiguousarray(xdt.astype(ml_dtypes.bfloat16)),
        a0=np.ascontiguousarray(coeffs[:, 0, 0:C].T),
        W1=W1, W1_3=W1 / 3.0, W1_m3=-W1 / 3.0, W1_m=-W1, W1_8=W1 / 8.0,
        I128=np.eye(H, dtype=np.float32),
        W2=inputs["W2"].astype(np.float32),
        W3s=W3s, b3c=b3c,
        b1r=inputs["b1"].reshape(1, H).astype(np.float32),
        b2r=inputs["b2"].reshape(1, H).astype(np.float32),
        Wi=np.ascontiguousarray(inputs["W_init"].astype(np.float32)),
        bic=inputs["b_init"].reshape(H, 1).astype(np.float32),
        Wo=inputs["W_out"].reshape(H, 1).astype(np.float32),
        bo=inputs["b_out"].reshape(1, 1).astype(np.float32),
        ones=np.ones((1, BC), np.float32),
    )


def kernel(**inputs):
    return _run(N_STEPS_DEFAULT, False, inputs)


def _run(n_steps, trace, inputs):
    from concourse.bass_utils import run_bass_kernel_spmd
    nc = build(n_steps)
    in_maps = [host_inputs(inputs, i, n_steps) for i in range(N_CORES)]
    res = run_bass_kernel_spmd(nc, in_maps, core_ids=list(range(N_CORES)),
                               trace=trace)
    out = np.concatenate([res.results[i]["out"] for i in range(N_CORES)],
                         axis=0)
    _run.last_result = res
    return out


# revision 3
# speedup vs baseline: 1.7244x; 1.7244x over previous
# Neural CDE kernel for 8 Trainium2 NeuronCores — v4.
# Data-parallel over batch: 4096 -> 512/core; 255-step RK4 scan local per core.
#
# v4: xd (spline derivative at the 3 RK4 eval points per step) is z-independent,
# so it is precomputed on the HOST into a DRAM table of [1, C*BC] bf16 rows and
# DMA-broadcast to [H, C*BC] SBUF tiles, prefetched 2 steps ahead on the SP
# queue. This removes the per-step SBUF->DRAM->broadcast staging chain and the
# sync-queue serialization that dominated v3. coeffs input is replaced by the
# xd table + pre-transposed a0.
#
# Retained from v3: full-width [H, 512] stages; c-einsum via tanh pieces (bf16)
# * xd (DVE) summed on PE via identity-matmul PSUM accumulation; RK4 y-combines
# folded into mm1 PSUM groups with scaled W1 copies; softplus = Abs, Exp (ACT)
# + fused DVE relu(x)+poly3(log1p(e)); layer-1 bias rides in u = W1 z + b1.
import sys
sys.path.insert(0, '/opt/trn_rl_repo')

import numpy as np

B_FULL, N_INT, C, H = 4096, 255, 8, 128
N_CORES = 8
B_CORE = B_FULL // N_CORES          # 512
BC = B_CORE
N_STEPS_DEFAULT = 255

# log1p(t) ~ c1 t + c2 t^2 + c3 t^3 on (0,1], max abs err 5.4e-4
SC1, SC2, SC3 = 0.9874542, -0.40841109, 0.11463897

# where the Abs of softplus runs: 'act' (scalar engine, v3 behavior),
# 'dve' (vector engine stt), 'pool' (gpsimd stt)
ABS_ENGINE = 'act'

_REGISTERED = {}


def _register_ops():
    if _REGISTERED:
        return _REGISTERED
    import concourse.dve_ops as dve_ops
    from concourse.dve_spec import (Spec, Src0, Src1, C0, C1, C2, Zero,
                                    maxx, lower, _has_src1)
    from concourse.dve_uop import DveOpSpec

    def reg(name, spec):
        if name in dve_ops._SUB_OPCODE_FOR_NAME:
            return next(o for o in dve_ops.OPS if o.name == name)
        shas = {}
        for ver in ("v3", "v4"):
            s = DveOpSpec(name=name, opcode=0, uops=lower(spec, ver=ver),
                          rd1_en=_has_src1(spec))
            shas[ver] = s.sha(ver)
        op = dve_ops.DveOp(name, spec, False, uops_sha=shas)
        dve_ops.OPS.append(op)
        dve_ops.CUSTOM_DVE_SPECS[name] = spec
        dve_ops._SUB_OPCODE_FOR_NAME[name] = max(
            dve_ops._SUB_OPCODE_FOR_NAME.values()) + 1
        return op

    def _sptail_ref(in0, in1, s0, s1, imm2):
        x = in0.astype(np.float32)
        t = in1.astype(np.float32)
        return np.maximum(x, 0) + ((imm2 * t + s1) * t + s0) * t

    _REGISTERED['SPTAIL3'] = reg(
        "SPTAIL3_ANT",
        Spec(body=maxx(Src0, Zero) + (((Src1 * C2 + C1) * Src1) + C0) * Src1,
             reference=_sptail_ref))
    return _REGISTERED


_NC_CACHE = {}


def build(n_steps=N_STEPS_DEFAULT):
    key = n_steps
    if key in _NC_CACHE:
        return _NC_CACHE[key]
    import concourse.mybir as mybir
    import concourse.tile as tile
    from concourse import bacc

    ops = _register_ops()
    SPTAIL3 = ops['SPTAIL3']

    f32 = mybir.dt.float32
    f32r = mybir.dt.float32r
    bf16 = mybir.dt.bfloat16
    AF = mybir.ActivationFunctionType
    OP = mybir.AluOpType
    OPm, OPa = OP.mult, OP.add

    nc = bacc.Bacc()
    # xd table: rows 3*s+j for j in {0:frac=1/3, 1:frac=2/3, 2:frac=1},
    # plus final row = xd(step 0, frac 0) for the very first k1.
    xdt_d = nc.dram_tensor("xdt", [3 * n_steps + 1, C * BC], bf16,
                           kind="ExternalInput")
    wd = {}
    for nm in ["W1", "W1_3", "W1_m3", "W1_m", "W1_8", "I128", "W2"]:
        wd[nm] = nc.dram_tensor(nm, [H, H], f32, kind="ExternalInput")
    W3s_d = nc.dram_tensor("W3s", [H, H * C], f32, kind="ExternalInput")
    b3c_d = nc.dram_tensor("b3c", [H, C], f32, kind="ExternalInput")
    b1r_d = nc.dram_tensor("b1r", [1, H], f32, kind="ExternalInput")
    b2r_d = nc.dram_tensor("b2r", [1, H], f32, kind="ExternalInput")
    a0_d = nc.dram_tensor("a0", [C, BC], f32, kind="ExternalInput")
    Wi_d = nc.dram_tensor("Wi", [C, H], f32, kind="ExternalInput")
    bi_d = nc.dram_tensor("bic", [H, 1], f32, kind="ExternalInput")
    Wo_d = nc.dram_tensor("Wo", [H, 1], f32, kind="ExternalInput")
    bo_d = nc.dram_tensor("bo", [1, 1], f32, kind="ExternalInput")
    ones_d = nc.dram_tensor("ones", [1, BC], f32, kind="ExternalInput")
    out_d = nc.dram_tensor("out", [BC, 1], f32, kind="ExternalOutput")

    with tile.TileContext(nc) as tc:
        with tc.tile_pool(name="const", bufs=1) as cp, \
             tc.tile_pool(name="zpool", bufs=2) as zp, \
             tc.tile_pool(name="work", bufs=2) as wp, \
             tc.tile_pool(name="tanh", bufs=6) as thp, \
             tc.tile_pool(name="mul", bufs=6) as mp, \
             tc.tile_pool(name="kpool", bufs=2) as kp, \
             tc.tile_pool(name="xdrep", bufs=4) as xp, \
             tc.tile_pool(name="psA", bufs=3, space="PSUM") as psA, \
             tc.tile_pool(name="psK", bufs=1, space="PSUM") as psK, \
             tc.tile_pool(name="ps3", bufs=4, space="PSUM") as ps3:

            # ---- constants ----
            W1_t = cp.tile([H, H], f32r, tag="w1")
            W1_3_t = cp.tile([H, H], f32r, tag="w13")
            W1_m3_t = cp.tile([H, H], f32r, tag="w1m3")
            W1_m_t = cp.tile([H, H], f32r, tag="w1m")
            W1_8_t = cp.tile([H, H], f32r, tag="w18")
            I_t = cp.tile([H, H], f32r, tag="ieye")
            Ib_t = cp.tile([H, H], bf16, tag="ibf")
            W2_t = cp.tile([H, H], f32r, tag="w2")
            W3_t = cp.tile([H, H * C], f32r, tag="w3")
            b3c_t = cp.tile([H, C], f32, tag="b3c")
            W1bf_t = cp.tile([H, H], bf16, tag="w1bf")
            W13bf_t = cp.tile([H, H], bf16, tag="w13bf")
            W18bf_t = cp.tile([H, H], bf16, tag="w18bf")
            b1r_t = cp.tile([1, H], f32r, tag="b1r")
            b2r_t = cp.tile([1, H], f32r, tag="b2r")
            Wi_t = cp.tile([C, H], f32r, tag="wi")
            Wo_t = cp.tile([H, 1], f32r, tag="wo")
            bi_t = cp.tile([H, 1], f32, tag="bi")
            bo_t = cp.tile([1, 1], f32, tag="bo")
            a0_t = cp.tile([C, BC], f32r, tag="a0")
            ones_t = cp.tile([1, BC], f32r, tag="ones")
            for t_, d_ in [(W1_t, wd["W1"]), (W1_3_t, wd["W1_3"]),
                           (W1_m3_t, wd["W1_m3"]), (W1_m_t, wd["W1_m"]),
                           (W1_8_t, wd["W1_8"]), (I_t, wd["I128"]),
                           (Ib_t, wd["I128"]), (W2_t, wd["W2"]),
                           (W3_t, W3s_d),
                           (W1bf_t, wd["W1"]), (W13bf_t, wd["W1_3"]),
                           (W18bf_t, wd["W1_8"]),
                           (b1r_t, b1r_d), (b2r_t, b2r_d), (Wi_t, Wi_d),
                           (Wo_t, Wo_d), (a0_t, a0_d)]:
                nc.gpsimd.dma_start(t_[:], d_[:])
            for t_, d_ in [(bi_t, bi_d), (bo_t, bo_d), (b3c_t, b3c_d)]:
                nc.sync.dma_start(t_[:], d_[:])
            nc.gpsimd.dma_start(ones_t[:], ones_d[:])

            # ---- z0 = a0 @ Wi + bi  (feature-major [H, B]) ----
            zps = psA.tile([H, BC], f32, tag="pA")
            nc.tensor.matmul(zps[:], Wi_t[:], a0_t[:], start=True, stop=True)
            zT = zp.tile([H, BC], f32r, tag="z")
            nc.scalar.activation(zT[:], zps[:], AF.Identity, bias=bi_t[:])

            # ---- xd loading: broadcast one DRAM row across 128 partitions ----
            def load_xd(row, tag):
                rep = xp.tile([H, C, BC], bf16, tag=tag)
                nc.sync.dma_start(
                    rep[:].rearrange("h s b -> h (s b)"),
                    xdt_d[row:row + 1, :].to_broadcast((H, C * BC)))
                return rep

            def load_step(step):
                return [load_xd(3 * step + j, f"x{j}") for j in range(3)]

            xd_prev = load_xd(3 * n_steps, "x0")
            pend = {0: load_step(0)}
            if n_steps > 1:
                pend[1] = load_step(1)

            # ---- u0 = W1 z0 + b1 ; p1 seeded for step-0 k1 ----
            p1_t = psA.tile([H, BC], f32, tag="pA", name="p1_init")
            nc.tensor.matmul(p1_t[:], b1r_t[:], ones_t[:],
                             start=True, stop=False)
            nc.tensor.matmul(p1_t[:], W1_t[:], zT[:], start=False, stop=True)
            u_t = wp.tile([H, BC], f32r, tag="u", name="u_init")
            nc.scalar.activation(u_t[:], p1_t[:], AF.Copy)

            def softplus_head(p1):
                """e = exp(-|p1|) on the chosen engine split."""
                e1 = wp.tile([H, BC], f32, tag="spe")
                if ABS_ENGINE == 'act':
                    a1 = wp.tile([H, BC], f32, tag="spa")
                    nc.scalar.activation(a1[:], p1[:], AF.Abs)
                    nc.scalar.activation(e1[:], a1[:], AF.Exp, scale=-1.0)
                else:
                    eng = nc.vector if ABS_ENGINE == 'dve' else nc.gpsimd
                    n1 = wp.tile([H, BC], f32, tag="spa")
                    eng.scalar_tensor_tensor(n1[:], p1[:], -1.0, p1[:],
                                             OPm, OP.min)
                    nc.scalar.activation(e1[:], n1[:], AF.Exp)
                return e1

            def tail(p1, xd_rep, j, feed=None):
                """softplus -> mm2 -> softplus -> mm3 (8 single-slot pieces)
                -> tanh -> mult (DVE) -> PE sum -> k."""
                e1 = softplus_head(p1)
                h1 = wp.tile([H, BC], f32r, tag="spha")
                nc.vector._custom_dve(SPTAIL3, out=h1[:], in0=p1[:],
                                      in1=e1[:], s0=SC1, s1=SC2, imm2=SC3)
                p2 = psA.tile([H, BC], f32, tag="pA")
                nc.tensor.matmul(p2[:], b2r_t[:], ones_t[:],
                                 start=True, stop=False)
                nc.tensor.matmul(p2[:], W2_t[:], h1[:], start=False, stop=True)
                e2 = softplus_head(p2)
                h2 = wp.tile([H, BC], f32r, tag="sphb")
                nc.vector._custom_dve(SPTAIL3, out=h2[:], in0=p2[:],
                                      in1=e2[:], s0=SC1, s1=SC2, imm2=SC3)
                # mm3: 8 single-slot pieces; b3 folded into tanh bias
                p3s = []
                for slot in range(C):
                    p3 = ps3.tile([H, BC], f32, tag="p3")
                    nc.tensor.matmul(p3[:],
                                     W3_t[:, slot * H:(slot + 1) * H], h2[:],
                                     start=True, stop=True)
                    p3s.append(p3)
                Ts = []
                for slot in range(C):
                    T_t = thp.tile([H, BC], bf16, tag="T")
                    nc.scalar.activation(T_t[:], p3s[slot][:], AF.Tanh,
                                         bias=b3c_t[:, slot:slot + 1])
                    Ts.append(T_t)
                pK = psK.tile([H, BC], f32, tag="pK")
                for slot in range(C):
                    P_t = mp.tile([H, BC], bf16, tag="P")
                    nc.vector.tensor_tensor(P_t[:], Ts[slot][:],
                                            xd_rep[:, slot, :], OPm)
                    nc.tensor.matmul(pK[:], Ib_t[:], P_t[:],
                                     start=(slot == 0), stop=(slot == C - 1))
                    if feed is not None:
                        Wf, pn = feed
                        nc.tensor.matmul(pn[:], Wf[:], P_t[:],
                                         start=False, stop=(slot == C - 1))
                k_t = kp.tile([H, BC], f32r, tag=f"k{j}")
                nc.vector.tensor_scalar_add(k_t[:], pK[:], 0.0)
                return k_t

            for step in range(n_steps):
                if step + 2 < n_steps:
                    pend[step + 2] = load_step(step + 2)
                xd13, xd23, xd1 = pend.pop(step)

                # k2's p1 group opens first: I u, then dirP W1/3 P(k1)
                pn2 = psA.tile([H, BC], f32, tag="pA", name="pn2")
                nc.tensor.matmul(pn2[:], I_t[:], u_t[:],
                                 start=True, stop=False)
                k1 = tail(p1_t, xd_prev, 1, feed=(W13bf_t, pn2))

                pn3 = psA.tile([H, BC], f32, tag="pA", name="pn3")
                nc.tensor.matmul(pn3[:], I_t[:], u_t[:],
                                 start=True, stop=False)
                nc.tensor.matmul(pn3[:], W1_m3_t[:], k1[:],
                                 start=False, stop=False)
                k2 = tail(pn2, xd13, 2, feed=(W1bf_t, pn3))

                w1 = wp.tile([H, BC], f32, tag="w1t")
                nc.vector.scalar_tensor_tensor(w1[:], k2[:], 3.0, k1[:],
                                               OPm, OPa)

                pn4 = psA.tile([H, BC], f32, tag="pA", name="pn4")
                nc.tensor.matmul(pn4[:], I_t[:], u_t[:],
                                 start=True, stop=False)
                nc.tensor.matmul(pn4[:], W1_t[:], k1[:],
                                 start=False, stop=False)
                nc.tensor.matmul(pn4[:], W1_m_t[:], k2[:],
                                 start=False, stop=False)
                k3 = tail(pn3, xd23, 3, feed=(W1bf_t, pn4))

                w2 = wp.tile([H, BC], f32r, tag="w2t")
                nc.vector.scalar_tensor_tensor(w2[:], k3[:], 3.0, w1[:],
                                               OPm, OPa)

                last = step == n_steps - 1
                if not last:
                    pn1 = psA.tile([H, BC], f32, tag="pA", name="pn1")
                    nc.tensor.matmul(pn1[:], I_t[:], u_t[:],
                                     start=True, stop=False)
                    nc.tensor.matmul(pn1[:], W1_8_t[:], w2[:],
                                     start=False, stop=False)
                    k4 = tail(pn4, xd1, 4, feed=(W18bf_t, pn1))
                else:
                    k4 = tail(pn4, xd1, 4)

                # z' = (z + w2/8) + k4/8
                zn = zp.tile([H, BC], f32r, tag="z")
                w3_ = wp.tile([H, BC], f32r, tag="w3t")
                nc.vector.scalar_tensor_tensor(w3_[:], w2[:], 0.125, zT[:],
                                               OPm, OPa)
                nc.vector.scalar_tensor_tensor(zn[:], k4[:], 0.125, w3_[:],
                                               OPm, OPa)
                if not last:
                    p1_t = pn1
                    u_t = wp.tile([H, BC], f32r, tag="u")
                    nc.scalar.activation(u_t[:], pn1[:], AF.Copy)
                zT = zn
                xd_prev = xd1

            # ---- out = zT @ W_out + b_out ----
            ops_ = psK.tile([H, BC], f32, tag="pK")
            nc.tensor.matmul(ops_[0:1, :], Wo_t[:], zT[:],
                             start=True, stop=True)
            ot = cp.tile([1, BC], f32, tag="outs")
            nc.scalar.activation(ot[:], ops_[0:1, :], AF.Identity,
                                 bias=bo_t[:])
            nc.sync.dma_start(
                out_d[:].rearrange("(p bh) one -> one (p bh)", p=H), ot[:])

    nc.finalize()
    _NC_CACHE[key] = nc
    return nc


def host_inputs(inputs, core, n_steps=N_STEPS_DEFAULT):
    import ml_dtypes
    coeffs = np.asarray(inputs["coeffs"][core * BC:(core + 1) * BC],
                        dtype=np.float32)          # [BC, N_INT, 4C]
    bco = coeffs[:, :, C:2 * C]                     # [BC, n, C]
    two_c = coeffs[:, :, 2 * C:3 * C]
    three_d = coeffs[:, :, 3 * C:4 * C]
    # xd rows: for step s, j in {0,1,2} -> frac (j+1)/3; last row: step0 frac0.
    fr = (np.arange(1, 4, dtype=np.float32) / 3.0)[None, :, None, None]
    xd = bco[:, None] + (two_c[:, None] + three_d[:, None] * fr) * fr
    # [BC, 3, n, C] -> rows [n*3, C*BC] with c-major, batch natural order
    xd = xd[:, :, :n_steps, :].transpose(2, 1, 3, 0)   # [n, 3, C, BC]
    xdt = np.concatenate(
        [xd.reshape(3 * n_steps, C * BC), bco[:, 0, :].T.reshape(1, C * BC)],
        axis=0)
    W1 = inputs["W1"].astype(np.float32)
    W3 = inputs["W3"].astype(np.float32)
    # W3s[h_in, slot*H + h_out] = W3[h_in, h_out*C + slot]
    W3s = np.ascontiguousarray(
        W3.reshape(H, H, C).transpose(0, 2, 1).reshape(H, H * C))
    b3c = np.ascontiguousarray(
        inputs["b3"].astype(np.float32).reshape(H, C))
    return dict(
        xdt=np.ascontiguousarray(xdt.astype(ml_dtypes.bfloat16)),
        a0=np.ascontiguousarray(coeffs[:, 0, 0:C].T),
        W1=W1, W1_3=W1 / 3.0, W1_m3=-W1 / 3.0, W1_m=-W1, W1_8=W1 / 8.0,
        I128=np.eye(H, dtype=np.float32),
        W2=inputs["W2"].astype(np.float32),
        W3s=W3s, b3c=b3c,
        b1r=inputs["b1"].reshape(1, H).astype(np.float32),
        b2r=inputs["b2"].reshape(1, H).astype(np.float32),
        Wi=np.ascontiguousarray(inputs["W_init"].astype(np.float32)),
        bic=inputs["b_init"].reshape(H, 1).astype(np.float32),
        Wo=inputs["W_out"].reshape(H, 1).astype(np.float32),
        bo=inputs["b_out"].reshape(1, 1).astype(np.float32),
        ones=np.ones((1, BC), np.float32),
    )


def kernel(**inputs):
    return _run(N_STEPS_DEFAULT, False, inputs)


def _run(n_steps, trace, inputs):
    from concourse.bass_utils import run_bass_kernel_spmd
    nc = build(n_steps)
    in_maps = [host_inputs(inputs, i, n_steps) for i in range(N_CORES)]
    res = run_bass_kernel_spmd(nc, in_maps, core_ids=list(range(N_CORES)),
                               trace=trace)
    out = np.concatenate([res.results[i]["out"] for i in range(N_CORES)],
                         axis=0)
    _run.last_result = res
    return out
